# revision 31
# baseline (speedup 1.0000x reference)
"""Trainium2 Bass kernel for nn_Attention_49924699848781.

Data-parallel over batch: core b computes batch element b (B=8 across the 8
NeuronCores). Per-core pipeline, all matmuls in float32r (fp32 storage, fp22
multiply, fp32 accumulate - full PE rate, vs 1/4 rate for true fp32):

  0. out2[t,o] = tgtT.T @ W2T + bias -> DRAM, computed up front on the
     freshly loaded tgtT tiles while phase A's other inputs stream in.
  A. scoresT[s,t] = sum_d src[s,d]*tgt[t,d]   ([S,T] layout so the softmax
     axis T - the reference softmaxes over dim=1 = target axis - lies along
     the free dimension), then masked softmax over t per s-row: add the
     host-encoded bf16 {0, -1e16} mask bias, negated free-axis max, ACT exp
     with per-partition bias and fused sum accumulation, reciprocal,
     normalize. Normalized weights stream to DRAM (also a kernel output).
  B. out1T[d,t] = sum_s src[s,d]*w[s,t]: lhsT = src in natural layout,
     rhs = w tiles re-read from DRAM (SBUF cannot hold w + everything
     else); contraction (si) is the outer loop so the first matmuls only
     need tiles that were already written early in phase A.
  C. out[t,o] = out1T.T @ W1T + out2 (re-read), K=D contraction with
     lhsT = out1T (SBUF-resident), rhs = host-pretransposed W1.

Host side pre-transposes src/tgt/W per core (pure layout), encodes the 0/1
mask as a bf16 additive bias, and rounds fp32 matmul operands to fp22
(13 mantissa bits) so the PE's float32r truncation becomes an unbiased
rounding. The w tiles for phase B's first chunk are prefetched during
phase A as soon as each wout row block lands in DRAM, so phase B starts
compute-bound. Measured on hardware: ~285 us per core (median; PE issues
1536 N=512 matmuls); output absmax rel err ~2e-3, softmax-weight output
~7e-3 (fp22 multiply precision).
"""

import sys

if "/opt/trn_rl_repo" not in sys.path:
    sys.path.insert(0, "/opt/trn_rl_repo")

import ml_dtypes
import numpy as np

import concourse.bass as bass
import concourse.bacc as bacc
import concourse.mybir as mybir
import concourse.tile as tile
from concourse.bass_utils import run_bass_kernel_spmd

B, S, T, D, O = 8, 2048, 2048, 1024, 1024
NEG_BIG = -1e16
N_CORES = 8

F32 = mybir.dt.float32
F32R = mybir.dt.float32r
I32 = mybir.dt.int32
I8 = mybir.dt.int8
BF16 = mybir.dt.bfloat16
AF = mybir.ActivationFunctionType
ALU = mybir.AluOpType


def build_program(s=S, t=T, d=D, o=O, reps=1):
    """One-core program; SPMD across 8 cores with different batch data.

    reps>1 repeats the whole body (barrier-separated) for timing runs:
    comparing pipelined dispatch rates of reps=1 vs reps=3 cancels the
    ~1.5 ms axon per-dispatch overhead.
    """
    nS, nKD, nTc, nKS = s // 128, d // 128, t // 512, s // 128
    nOc, nTm = o // 512, t // 128
    nPre = min(nS, nKS // 2)  # srcN tiles prefetched during phase A

    nc = bacc.Bacc(target_bir_lowering=False)

    srcT = nc.dram_tensor("srcT", [d, s], F32R, kind="ExternalInput")
    srcN = nc.dram_tensor("srcN", [s, d], F32R, kind="ExternalInput")
    tgtT = nc.dram_tensor("tgtT", [d, t], F32R, kind="ExternalInput")
    maskT = nc.dram_tensor("maskT", [s, t], BF16, kind="ExternalInput")
    w1t = nc.dram_tensor("w1t", [d, o], F32R, kind="ExternalInput")
    w2t = nc.dram_tensor("w2t", [d, o], F32R, kind="ExternalInput")
    biasd = nc.dram_tensor("biasd", [1, o], F32, kind="ExternalInput")
    wout = nc.dram_tensor("wout", [s, t], F32R, kind="ExternalOutput")
    out2d = nc.dram_tensor("out2d", [t, o], F32)
    outd = nc.dram_tensor("outd", [t, o], F32, kind="ExternalOutput")

    with tile.TileContext(
        nc, pool_alloc_mode="queue" if reps == 1 else "stack"
    ) as tc:
        for _rep in range(reps):
            if _rep:
                tc.strict_bb_all_engine_barrier()
            with (
                tc.tile_pool(name="srcNp", bufs=1) as srcN_pool,
                tc.tile_pool(name="biasp", bufs=1) as bias_pool,
                tc.tile_pool(name="wB", bufs=16) as wB_pool,
            ):
                srcN_tiles = [None] * nKS
                wchunks = {}

                def load_srcN(si, pool):
                    sn = pool.tile([128, d], F32R, tag=f"srcN{si}", name=f"srcN{si}")
                    nc.sync.dma_start(out=sn, in_=srcN[si * 128 : (si + 1) * 128, :])
                    srcN_tiles[si] = sn

                bias_sb = bias_pool.tile([128, o], F32, tag="bias")
                bap = biasd[0:1, :]
                nc.sync.dma_start(
                    out=bias_sb,
                    in_=bass.AP(
                        tensor=bap.tensor, offset=bap.offset, ap=[[0, 128], [1, o]]
                    ),
                )
                wout_writes = []

                def load_wb(n, si):
                    wb = wB_pool.tile(
                        [128, 512], F32R, tag="wb", name=f"wb_{n}_{si}"
                    )
                    rd = nc.sync.dma_start(
                        out=wb,
                        in_=wout[
                            si * 128 : (si + 1) * 128, n * 512 : (n + 1) * 512
                        ],
                    )
                    tile.add_dep_helper(
                        rd.ins, wout_writes[si].ins, sync=True,
                        reason="wout DRAM RAW",
                    )
                    wchunks.setdefault(n, {})[si] = wb

                # ---- phase A: scoresT [s,t] + softmax over t (free axis) ----
                with (
                    tc.tile_pool(name="tgtA", bufs=1) as tgtA_pool,
                    tc.tile_pool(name="srcTp", bufs=2) as srcT_pool,
                    tc.tile_pool(name="maskp", bufs=2) as mask_pool,
                    tc.tile_pool(name="mdp", bufs=2) as md_pool,
                    tc.tile_pool(name="wp", bufs=2) as w_pool,
                    tc.tile_pool(name="stats", bufs=8) as stats_pool,
                ):
                    tgtA_tiles = [None] * nKD

                    def load_tgtA(k):
                        tt = tgtA_pool.tile(
                            [128, t], F32R, tag=f"tgtA{k}", name=f"tgtA{k}"
                        )
                        nc.sync.dma_start(
                            out=tt, in_=tgtT[k * 128 : (k + 1) * 128, :]
                        )
                        tgtA_tiles[k] = tt

                    # ---- phase 0: out2[t,o] = tgtT.T @ W2T + bias -> DRAM.
                    # Runs on freshly-loaded tgtA tiles while phase A's other
                    # input streams warm up; phase C later adds out1@W1T.
                    out2_writes = {}
                    with (
                        tc.tile_pool(name="w2p", bufs=1) as w2_pool,
                        tc.tile_pool(name="o2o", bufs=2) as o2o_pool,
                        tc.tile_pool(name="ps0", bufs=2, space="PSUM") as ps0_pool,
                    ):
                        for oc in range(nOc):
                            w2k = []
                            for k in range(nKD):
                                wt = w2_pool.tile(
                                    [128, 512], F32R, tag=f"w2b{k}", name=f"w2b{k}"
                                )
                                nc.sync.dma_start(
                                    out=wt,
                                    in_=w2t[
                                        k * 128 : (k + 1) * 128,
                                        oc * 512 : (oc + 1) * 512,
                                    ],
                                )
                                w2k.append(wt)
                                if tgtA_tiles[k] is None:
                                    load_tgtA(k)
                            for tm in range(nTm):
                                if tm % 4 == 0:
                                    p0 = ps0_pool.tile(
                                        [128, 4, 512], F32, tag="ps0",
                                        name=f"ps0_{oc}_{tm}",
                                    )
                                for k in range(nKD):
                                    nc.tensor.matmul(
                                        p0[:, tm % 4, :],
                                        tgtA_tiles[k][:, tm * 128 : (tm + 1) * 128],
                                        w2k[k],
                                        start=(k == 0),
                                        stop=(k == nKD - 1),
                                    )
                                o2t = o2o_pool.tile([128, 512], F32, tag="o2t")
                                nc.vector.tensor_add(
                                    o2t, p0[:, tm % 4, :],
                                    bias_sb[:, oc * 512 : (oc + 1) * 512],
                                )
                                out2_writes[(oc, tm)] = nc.scalar.dma_start(
                                    out=out2d[
                                        tm * 128 : (tm + 1) * 128,
                                        oc * 512 : (oc + 1) * 512,
                                    ],
                                    in_=o2t,
                                )

                    psA_cm = tc.tile_pool(name="psA", bufs=2, space="PSUM")
                    psA_pool = psA_cm.__enter__()
                    for si in range(nS):
                        st = srcT_pool.tile([128, nKD, 128], F32R, tag="st")
                        nc.sync.dma_start(
                            out=st,
                            in_=srcT[:, si * 128 : (si + 1) * 128].rearrange(
                                "(k p) m -> p k m", p=128
                            ),
                        )
                        # host pre-encodes mask as bf16 {0, NEG_BIG} additive bias
                        mk = mask_pool.tile([128, t], BF16, tag="mk")
                        nc.scalar.dma_start(
                            out=mk, in_=maskT[si * 128 : (si + 1) * 128, :]
                        )

                        ps = psA_pool.tile([128, t], F32, tag="psA")
                        for k in range(nKD):
                            if tgtA_tiles[k] is None:
                                load_tgtA(k)
                            lw = st[:, k, :]
                            for n in range(nTc):
                                nc.tensor.matmul(
                                    ps[:, n * 512 : (n + 1) * 512],
                                    lw,
                                    tgtA_tiles[k][:, n * 512 : (n + 1) * 512],
                                    start=(k == 0),
                                    stop=(k == nKD - 1),
                                )

                        md = md_pool.tile([128, t], F32, tag="md")
                        nc.vector.tensor_add(md, ps, mk)
                        nmax = stats_pool.tile([128, 1], F32, tag="nmax")
                        nc.vector.tensor_reduce(
                            nmax, md, axis=mybir.AxisListType.X, op=ALU.max,
                            negate=True,
                        )
                        ex = w_pool.tile([128, t], F32R, tag="ex")
                        zz = stats_pool.tile([128, 1], F32, tag="zz")
                        # ex = exp(masked - max) ; zz = sum_t(ex)
                        nc.scalar.activation(
                            out=ex, in_=md, func=AF.Exp, bias=nmax, scale=1.0,
                            accum_out=zz,
                        )
                        rz = stats_pool.tile([128, 1], F32, tag="rz")
                        nc.vector.reciprocal(rz, zz)
                        nc.vector.tensor_scalar_mul(ex, ex, rz)
                        wout_writes.append(
                            nc.scalar.dma_start(
                                out=wout[si * 128 : (si + 1) * 128, :], in_=ex
                            )
                        )
                        load_wb(0, si)
                        if si < nPre:
                            load_srcN(si, srcN_pool)
                    psA_cm.__exit__(None, None, None)

                # wout round-trips through DRAM into phase B. Tile does not
                # track DRAM RAW hazards, so each wB read carries an explicit
                # sync dep on the wout tile write it consumes.
                # -------- phase B: out1T[d,t] = sum_s src[s,d] * w[s,t] ------
                with tc.tile_pool(name="out1", bufs=1) as out1_pool:
                    out1_tiles = []
                    for k in range(nKD):
                        out1_tiles.append(
                            out1_pool.tile(
                                [128, t], F32R, tag=f"out1_{k}", name=f"out1_{k}"
                            )
                        )

                    with (
                        tc.tile_pool(name="srcNb", bufs=1) as srcNb_pool,
                        tc.tile_pool(name="psB", bufs=2, space="PSUM") as psB_pool,
                    ):
                        for si in range(nKS):
                            if si >= nPre:
                                load_srcN(si, srcNb_pool)

                        dgrp = min(4, nKD)
                        nDh = nKD // dgrp
                        for n in range(nTc):
                            if n not in wchunks:
                                for si in range(nKS):
                                    load_wb(n, si)
                            wchunk = [wchunks[n][si] for si in range(nKS)]
                            # si-outer: the first matmuls need only w/src tile
                            # si=0, which is ready while phase A is still
                            # finishing the tail rows -> no A->B stall.
                            pss = []
                            for dh in range(nDh):
                                pss.append(
                                    psB_pool.tile(
                                        [128, dgrp, 512], F32, tag="psB",
                                        name=f"psB_{n}_{dh}",
                                    )
                                )
                            for si in range(nKS):
                                for dh in range(nDh):
                                    for d4 in range(dgrp):
                                        dd = dh * dgrp + d4
                                        nc.tensor.matmul(
                                            pss[dh][:, d4, :],
                                            srcN_tiles[si][
                                                :, dd * 128 : (dd + 1) * 128
                                            ],
                                            wchunk[si],
                                            start=(si == 0),
                                            stop=(si == nKS - 1),
                                        )
                            for dh in range(nDh):
                                for d4 in range(dgrp):
                                    dd = dh * dgrp + d4
                                    nc.vector.tensor_copy(
                                        out1_tiles[dd][:, n * 512 : (n + 1) * 512],
                                        pss[dh][:, d4, :],
                                    )

                    # ------ phase C: out[t,o] = out1T.T @ W1T + out2 --------
                    with (
                        tc.tile_pool(name="wgt1", bufs=2) as wgt1_pool,
                        tc.tile_pool(name="o2r", bufs=4) as o2r_pool,
                        tc.tile_pool(name="outC", bufs=4) as outC_pool,
                        tc.tile_pool(name="psC", bufs=2, space="PSUM") as psC_pool,
                    ):
                        for oc in range(nOc):
                            w1b = wgt1_pool.tile([128, nKD, 512], F32R, tag="w1b")
                            nc.sync.dma_start(
                                out=w1b,
                                in_=w1t[:, oc * 512 : (oc + 1) * 512].rearrange(
                                    "(k p) m -> p k m", p=128
                                ),
                            )
                            for tm in range(nTm):
                                o2t = o2r_pool.tile([128, 512], F32, tag="o2t")
                                rd = nc.scalar.dma_start(
                                    out=o2t,
                                    in_=out2d[
                                        tm * 128 : (tm + 1) * 128,
                                        oc * 512 : (oc + 1) * 512,
                                    ],
                                )
                                tile.add_dep_helper(
                                    rd.ins,
                                    out2_writes[(oc, tm)].ins,
                                    sync=True,
                                    reason="out2 DRAM RAW",
                                )
                                if tm % 4 == 0:
                                    ps = psC_pool.tile(
                                        [128, 4, 512], F32, tag="psC",
                                        name=f"psC_{oc}_{tm}",
                                    )
                                for k in range(nKD):
                                    nc.tensor.matmul(
                                        ps[:, tm % 4, :],
                                        out1_tiles[k][:, tm * 128 : (tm + 1) * 128],
                                        w1b[:, k, :],
                                        start=(k == 0),
                                        stop=(k == nKD - 1),
                                    )
                                oc_t = outC_pool.tile([128, 512], F32, tag="oct")
                                nc.vector.tensor_add(oc_t, ps[:, tm % 4, :], o2t)
                                nc.scalar.dma_start(
                                    out=outd[
                                        tm * 128 : (tm + 1) * 128,
                                        oc * 512 : (oc + 1) * 512,
                                    ],
                                    in_=oc_t,
                                )

    nc.finalize()
    return nc


def _r22(x):
    """Round fp32 to fp22 (13 explicit mantissa bits) so the PE's float32r
    truncation is exact -> unbiased vs the fp32 reference."""
    u = np.ascontiguousarray(x, dtype=np.float32).view(np.uint32)
    u = (u + np.uint32(0x200)) & np.uint32(0xFFFFFC00)
    return u.view(np.float32)


def make_in_maps(src, tgt, mask, W, b):
    src = np.asarray(src, dtype=np.float32)
    tgt = np.asarray(tgt, dtype=np.float32)
    mask = np.asarray(mask, dtype=np.int32)
    W = np.asarray(W, dtype=np.float32)
    b = np.asarray(b, dtype=np.float32)

    w1t_ = _r22(np.ascontiguousarray(W[:, :D].T))
    w2t_ = _r22(np.ascontiguousarray(W[:, D:].T))
    bias_ = np.ascontiguousarray(b.reshape(1, O))

    in_maps = []
    for bi in range(B):
        in_maps.append(
            {
                "srcT": _r22(np.ascontiguousarray(src[bi].T)),
                "srcN": _r22(src[bi]),
                "tgtT": _r22(np.ascontiguousarray(tgt[bi].T)),
                "maskT": np.ascontiguousarray(
                    mask[:, bi, :].T.astype(np.float32) * np.float32(NEG_BIG)
                ).astype(ml_dtypes.bfloat16),
                "w1t": w1t_,
                "w2t": w2t_,
                "biasd": bias_,
            }
        )
    return in_maps


_PROGRAM = None


def _get_program():
    global _PROGRAM
    if _PROGRAM is None:
        _PROGRAM = build_program()
    return _PROGRAM


def run(inputs, trace=False):
    nc = _get_program()
    in_maps = make_in_maps(**inputs)
    res = run_bass_kernel_spmd(nc, in_maps, list(range(N_CORES)), trace=trace)
    output = np.stack([res.results[i]["outd"] for i in range(B)])
    weight = np.stack([np.ascontiguousarray(res.results[i]["wout"].T) for i in range(B)])
    return (output, weight), res


def kernel(**inputs):
    outs, _ = run(inputs, trace=False)
    return outs


# ---------------------------------------------------------------- benchmarking
def bench(inputs, iters=8, reps=1, k_pipe=16):
    """Steady-state per-dispatch timing with device-resident inputs.

    Compiles the SPMD program once, stages all per-core inputs on the 8
    devices, then times repeated executions (no donation, so buffers stay
    valid). Returns (sorted per-call seconds, outputs_of_last_call).
    """
    import time

    import jax
    from jax.experimental.shard_map import shard_map
    from jax.sharding import Mesh, PartitionSpec

    from concourse import bass2jax as b2j

    b2j.install_neuronx_cc_hook()

    nc = _get_program() if reps == 1 else build_program(reps=reps)
    in_maps = make_in_maps(**inputs)
    n_cores = N_CORES

    in_names, out_names, out_avals, zero_outs = [], [], [], []
    partition_name = nc.partition_id_tensor.name if nc.partition_id_tensor else None
    for alloc in nc.m.functions[0].allocations:
        if not isinstance(alloc, mybir.MemoryLocationSet):
            continue
        name = alloc.memorylocations[0].name
        if alloc.kind == "ExternalInput":
            if name != partition_name:
                in_names.append(name)
        elif alloc.kind == "ExternalOutput":
            out_names.append(name)
            shape = tuple(alloc.tensor_shape)
            dtype = mybir.dt.np(alloc.dtype)
            out_avals.append(jax.core.ShapedArray(shape, dtype))
            zero_outs.append(np.zeros(shape, dtype))
    n_params = len(in_names)
    all_in_names = list(in_names) + list(out_names)
    if partition_name is not None:
        all_in_names.append(partition_name)

    def _body(*args):
        operands = list(args)
        if partition_name is not None:
            operands.append(b2j.partition_id_tensor())
        outs = b2j._bass_exec_p.bind(
            *operands,
            out_avals=tuple(out_avals),
            in_names=tuple(all_in_names),
            out_names=tuple(out_names),
            lowering_input_output_aliases=(),
            sim_require_finite=True,
            sim_require_nnan=True,
            nc=nc,
        )
        return tuple(outs)

    devices = jax.devices()[:n_cores]
    mesh = Mesh(np.asarray(devices), ("core",))
    in_specs = (PartitionSpec("core"),) * (n_params + len(out_names))
    out_specs = (PartitionSpec("core"),) * len(out_names)
    sharded = jax.jit(
        shard_map(_body, mesh=mesh, in_specs=in_specs, out_specs=out_specs,
                  check_rep=False),
        keep_unused=True,
    )
    sharding = jax.sharding.NamedSharding(mesh, PartitionSpec("core"))
    concat_in = [
        jax.device_put(
            np.concatenate([np.asarray(in_maps[c][nm]) for c in range(n_cores)], 0),
            sharding,
        )
        for nm in in_names
    ]
    concat_zeros = [
        jax.device_put(np.zeros((n_cores * z.shape[0], *z.shape[1:]), z.dtype), sharding)
        for z in zero_outs
    ]
    # warmup (compile + first exec)
    outs = sharded(*concat_in, *concat_zeros)
    jax.block_until_ready(outs)
    times = []
    for _ in range(iters):
        t0 = time.perf_counter()
        outs = sharded(*concat_in, *concat_zeros)
        jax.block_until_ready(outs)
        times.append(time.perf_counter() - t0)
    # pipelined: K async dispatches, block once -> slope amortizes RPC RTT
    K_PIPE = k_pipe
    jax.block_until_ready(sharded(*concat_in, *concat_zeros))
    t0 = time.perf_counter()
    last = None
    for _ in range(K_PIPE):
        last = sharded(*concat_in, *concat_zeros)
    jax.block_until_ready(last)
    t_k = time.perf_counter() - t0
    t0 = time.perf_counter()
    jax.block_until_ready(sharded(*concat_in, *concat_zeros))
    t_1 = time.perf_counter() - t0
    pipe_slope = (t_k - t_1) / (K_PIPE - 1)
    results = [
        {nm: np.asarray(outs[i]).reshape(n_cores, *out_avals[i].shape)[c]
         for i, nm in enumerate(out_names)}
        for c in range(n_cores)
    ]
    return (sorted(times), pipe_slope, t_k, t_1), results


# revision 32
# speedup vs baseline: 1.2340x; 1.2340x over previous
"""Trainium2 Bass kernel for nn_Attention_49924699848781.

Data-parallel over batch: core b computes batch element b (B=8 across the 8
NeuronCores). Per-core pipeline, all matmuls in float32r (fp32 storage, fp22
multiply, fp32 accumulate - full PE rate, vs 1/4 rate for true fp32):

  0. out2[t,o] = tgtT.T @ W2T + bias -> DRAM, computed up front on the
     freshly loaded tgtT tiles while phase A's other inputs stream in.
  A. scoresT[s,t] = sum_d src[s,d]*tgt[t,d]   ([S,T] layout so the softmax
     axis T - the reference softmaxes over dim=1 = target axis - lies along
     the free dimension), then masked softmax over t per s-row: add the
     host-encoded bf16 {0, -1e16} mask bias, negated free-axis max, ACT exp
     with per-partition bias and fused sum accumulation, reciprocal,
     normalize. Normalized weights stream to DRAM (also a kernel output).
  B. out1T[d,t] = sum_s src[s,d]*w[s,t]: lhsT = src in natural layout,
     rhs = w tiles re-read from DRAM (SBUF cannot hold w + everything
     else); contraction (si) is the outer loop so the first matmuls only
     need tiles that were already written early in phase A.
  C. out[t,o] = out1T.T @ W1T + out2 (re-read), K=D contraction with
     lhsT = out1T (SBUF-resident), rhs = host-pretransposed W1.

Host side pre-transposes src/tgt/W per core (pure layout), encodes the 0/1
mask as a bf16 additive bias, and rounds fp32 matmul operands to fp22
(13 mantissa bits) so the PE's float32r truncation becomes an unbiased
rounding. The w tiles for phase B's first chunk are prefetched during
phase A as soon as each wout row block lands in DRAM, so phase B starts
compute-bound. Measured on hardware: ~285 us per core (median; PE issues
1536 N=512 matmuls); output absmax rel err ~2e-3, softmax-weight output
~7e-3 (fp22 multiply precision).
"""

import sys

if "/opt/trn_rl_repo" not in sys.path:
    sys.path.insert(0, "/opt/trn_rl_repo")

import ml_dtypes
import numpy as np

import concourse.bass as bass
import concourse.bacc as bacc
import concourse.mybir as mybir
import concourse.tile as tile
from concourse.bass_utils import run_bass_kernel_spmd

B, S, T, D, O = 8, 2048, 2048, 1024, 1024
NEG_BIG = -1e16
N_CORES = 8

F32 = mybir.dt.float32
F32R = mybir.dt.float32r
I32 = mybir.dt.int32
I8 = mybir.dt.int8
BF16 = mybir.dt.bfloat16
AF = mybir.ActivationFunctionType
ALU = mybir.AluOpType


def build_program(s=S, t=T, d=D, o=O, reps=1):
    """One-core program; SPMD across 8 cores with different batch data.

    reps>1 repeats the whole body (barrier-separated) for timing runs:
    comparing pipelined dispatch rates of reps=1 vs reps=3 cancels the
    ~1.5 ms axon per-dispatch overhead.
    """
    nS, nKD, nTc, nKS = s // 128, d // 128, t // 512, s // 128
    nOc, nTm = o // 512, t // 128
    nPre = min(nS, nKS // 2)  # srcN tiles prefetched during phase A

    nc = bacc.Bacc(target_bir_lowering=False)

    srcT = nc.dram_tensor("srcT", [d, s], F32R, kind="ExternalInput")
    srcN = nc.dram_tensor("srcN", [s, d], F32R, kind="ExternalInput")
    tgtT = nc.dram_tensor("tgtT", [d, t], F32R, kind="ExternalInput")
    maskT = nc.dram_tensor("maskT", [s, t], BF16, kind="ExternalInput")
    w1t = nc.dram_tensor("w1t", [d, o], F32R, kind="ExternalInput")
    w2t = nc.dram_tensor("w2t", [d, o], F32R, kind="ExternalInput")
    biasd = nc.dram_tensor("biasd", [1, o], F32, kind="ExternalInput")
    wout = nc.dram_tensor("wout", [s, t], F32R, kind="ExternalOutput")
    out2d = nc.dram_tensor("out2d", [t, o], F32)
    outd = nc.dram_tensor("outd", [t, o], F32, kind="ExternalOutput")

    with tile.TileContext(
        nc, pool_alloc_mode="queue" if reps == 1 else "stack"
    ) as tc:
        for _rep in range(reps):
            if _rep:
                tc.strict_bb_all_engine_barrier()
            with (
                tc.tile_pool(name="srcNp", bufs=1) as srcN_pool,
                tc.tile_pool(name="biasp", bufs=1) as bias_pool,
                tc.tile_pool(name="wB", bufs=16) as wB_pool,
            ):
                srcN_tiles = [None] * nKS
                wchunks = {}

                def load_srcN(si, pool):
                    sn = pool.tile([128, d], F32R, tag=f"srcN{si}", name=f"srcN{si}")
                    nc.sync.dma_start(out=sn, in_=srcN[si * 128 : (si + 1) * 128, :])
                    srcN_tiles[si] = sn

                bias_sb = bias_pool.tile([128, o], F32, tag="bias")
                bap = biasd[0:1, :]
                nc.sync.dma_start(
                    out=bias_sb,
                    in_=bass.AP(
                        tensor=bap.tensor, offset=bap.offset, ap=[[0, 128], [1, o]]
                    ),
                )
                wout_writes = []

                def load_wb(n, si):
                    wb = wB_pool.tile(
                        [128, 512], F32R, tag="wb", name=f"wb_{n}_{si}"
                    )
                    rd = nc.sync.dma_start(
                        out=wb,
                        in_=wout[
                            si * 128 : (si + 1) * 128, n * 512 : (n + 1) * 512
                        ],
                    )
                    tile.add_dep_helper(
                        rd.ins, wout_writes[si].ins, sync=True,
                        reason="wout DRAM RAW",
                    )
                    wchunks.setdefault(n, {})[si] = wb

                # ---- phase A: scoresT [s,t] + softmax over t (free axis) ----
                with (
                    tc.tile_pool(name="tgtA", bufs=1) as tgtA_pool,
                    tc.tile_pool(name="srcTp", bufs=2) as srcT_pool,
                    tc.tile_pool(name="maskp", bufs=2) as mask_pool,
                    tc.tile_pool(name="mdp", bufs=2) as md_pool,
                    tc.tile_pool(name="wp", bufs=2) as w_pool,
                    tc.tile_pool(name="stats", bufs=8) as stats_pool,
                ):
                    tgtA_tiles = [None] * nKD

                    def load_tgtA(k):
                        tt = tgtA_pool.tile(
                            [128, t], F32R, tag=f"tgtA{k}", name=f"tgtA{k}"
                        )
                        nc.sync.dma_start(
                            out=tt, in_=tgtT[k * 128 : (k + 1) * 128, :]
                        )
                        tgtA_tiles[k] = tt

                    # ---- phase 0: out2[t,o] = tgtT.T @ W2T + bias -> DRAM.
                    # Runs on freshly-loaded tgtA tiles while phase A's other
                    # input streams warm up; phase C later adds out1@W1T.
                    out2_writes = {}
                    with (
                        tc.tile_pool(name="w2p", bufs=1) as w2_pool,
                        tc.tile_pool(name="o2o", bufs=2) as o2o_pool,
                        tc.tile_pool(name="ps0", bufs=2, space="PSUM") as ps0_pool,
                    ):
                        for oc in range(nOc):
                            w2k = []
                            for k in range(nKD):
                                wt = w2_pool.tile(
                                    [128, 512], F32R, tag=f"w2b{k}", name=f"w2b{k}"
                                )
                                nc.sync.dma_start(
                                    out=wt,
                                    in_=w2t[
                                        k * 128 : (k + 1) * 128,
                                        oc * 512 : (oc + 1) * 512,
                                    ],
                                )
                                w2k.append(wt)
                                if tgtA_tiles[k] is None:
                                    load_tgtA(k)
                            for tm in range(nTm):
                                p0 = ps0_pool.tile([128, 512], F32, tag="ps0")
                                for k in range(nKD):
                                    nc.tensor.matmul(
                                        p0,
                                        tgtA_tiles[k][:, tm * 128 : (tm + 1) * 128],
                                        w2k[k],
                                        start=(k == 0),
                                        stop=(k == nKD - 1),
                                    )
                                o2t = o2o_pool.tile([128, 512], F32, tag="o2t")
                                nc.vector.tensor_add(
                                    o2t, p0, bias_sb[:, oc * 512 : (oc + 1) * 512]
                                )
                                out2_writes[(oc, tm)] = nc.scalar.dma_start(
                                    out=out2d[
                                        tm * 128 : (tm + 1) * 128,
                                        oc * 512 : (oc + 1) * 512,
                                    ],
                                    in_=o2t,
                                )

                    psA_cm = tc.tile_pool(name="psA", bufs=2, space="PSUM")
                    psA_pool = psA_cm.__enter__()
                    for si in range(nS):
                        st = srcT_pool.tile([128, nKD, 128], F32R, tag="st")
                        nc.sync.dma_start(
                            out=st,
                            in_=srcT[:, si * 128 : (si + 1) * 128].rearrange(
                                "(k p) m -> p k m", p=128
                            ),
                        )
                        # host pre-encodes mask as bf16 {0, NEG_BIG} additive bias
                        mk = mask_pool.tile([128, t], BF16, tag="mk")
                        nc.scalar.dma_start(
                            out=mk, in_=maskT[si * 128 : (si + 1) * 128, :]
                        )

                        ps = psA_pool.tile([128, t], F32, tag="psA")
                        for k in range(nKD):
                            if tgtA_tiles[k] is None:
                                load_tgtA(k)
                            lw = st[:, k, :]
                            for n in range(nTc):
                                nc.tensor.matmul(
                                    ps[:, n * 512 : (n + 1) * 512],
                                    lw,
                                    tgtA_tiles[k][:, n * 512 : (n + 1) * 512],
                                    start=(k == 0),
                                    stop=(k == nKD - 1),
                                )

                        md = md_pool.tile([128, t], F32, tag="md")
                        nc.vector.tensor_add(md, ps, mk)
                        nmax = stats_pool.tile([128, 1], F32, tag="nmax")
                        nc.vector.tensor_reduce(
                            nmax, md, axis=mybir.AxisListType.X, op=ALU.max,
                            negate=True,
                        )
                        ex = w_pool.tile([128, t], F32R, tag="ex")
                        zz = stats_pool.tile([128, 1], F32, tag="zz")
                        # ex = exp(masked - max) ; zz = sum_t(ex)
                        nc.scalar.activation(
                            out=ex, in_=md, func=AF.Exp, bias=nmax, scale=1.0,
                            accum_out=zz,
                        )
                        rz = stats_pool.tile([128, 1], F32, tag="rz")
                        nc.vector.reciprocal(rz, zz)
                        nc.vector.tensor_scalar_mul(ex, ex, rz)
                        wout_writes.append(
                            nc.scalar.dma_start(
                                out=wout[si * 128 : (si + 1) * 128, :], in_=ex
                            )
                        )
                        load_wb(0, si)
                        if si < nPre:
                            load_srcN(si, srcN_pool)
                    psA_cm.__exit__(None, None, None)

                # wout round-trips through DRAM into phase B. Tile does not
                # track DRAM RAW hazards, so each wB read carries an explicit
                # sync dep on the wout tile write it consumes.
                # -------- phase B: out1T[d,t] = sum_s src[s,d] * w[s,t] ------
                with tc.tile_pool(name="out1", bufs=1) as out1_pool:
                    out1_tiles = []
                    for k in range(nKD):
                        out1_tiles.append(
                            out1_pool.tile(
                                [128, t], F32R, tag=f"out1_{k}", name=f"out1_{k}"
                            )
                        )

                    with (
                        tc.tile_pool(name="srcNb", bufs=1) as srcNb_pool,
                        tc.tile_pool(name="psB", bufs=2, space="PSUM") as psB_pool,
                    ):
                        for si in range(nKS):
                            if si >= nPre:
                                load_srcN(si, srcNb_pool)

                        dgrp = min(4, nKD)
                        nDh = nKD // dgrp
                        for n in range(nTc):
                            if n not in wchunks:
                                for si in range(nKS):
                                    load_wb(n, si)
                            wchunk = [wchunks[n][si] for si in range(nKS)]
                            # si-outer: the first matmuls need only w/src tile
                            # si=0, which is ready while phase A is still
                            # finishing the tail rows -> no A->B stall.
                            pss = []
                            for dh in range(nDh):
                                pss.append(
                                    psB_pool.tile(
                                        [128, dgrp, 512], F32, tag="psB",
                                        name=f"psB_{n}_{dh}",
                                    )
                                )
                            for si in range(nKS):
                                for dh in range(nDh):
                                    for d4 in range(dgrp):
                                        dd = dh * dgrp + d4
                                        nc.tensor.matmul(
                                            pss[dh][:, d4, :],
                                            srcN_tiles[si][
                                                :, dd * 128 : (dd + 1) * 128
                                            ],
                                            wchunk[si],
                                            start=(si == 0),
                                            stop=(si == nKS - 1),
                                        )
                            for dh in range(nDh):
                                for d4 in range(dgrp):
                                    dd = dh * dgrp + d4
                                    nc.vector.tensor_copy(
                                        out1_tiles[dd][:, n * 512 : (n + 1) * 512],
                                        pss[dh][:, d4, :],
                                    )

                    # ------ phase C: out[t,o] = out1T.T @ W1T + out2 --------
                    with (
                        tc.tile_pool(name="wgt1", bufs=2) as wgt1_pool,
                        tc.tile_pool(name="o2r", bufs=4) as o2r_pool,
                        tc.tile_pool(name="outC", bufs=4) as outC_pool,
                        tc.tile_pool(name="psC", bufs=4, space="PSUM") as psC_pool,
                    ):
                        for oc in range(nOc):
                            w1b = wgt1_pool.tile([128, nKD, 512], F32R, tag="w1b")
                            nc.sync.dma_start(
                                out=w1b,
                                in_=w1t[:, oc * 512 : (oc + 1) * 512].rearrange(
                                    "(k p) m -> p k m", p=128
                                ),
                            )
                            for tm in range(nTm):
                                o2t = o2r_pool.tile([128, 512], F32, tag="o2t")
                                rd = nc.scalar.dma_start(
                                    out=o2t,
                                    in_=out2d[
                                        tm * 128 : (tm + 1) * 128,
                                        oc * 512 : (oc + 1) * 512,
                                    ],
                                )
                                tile.add_dep_helper(
                                    rd.ins,
                                    out2_writes[(oc, tm)].ins,
                                    sync=True,
                                    reason="out2 DRAM RAW",
                                )
                                ps = psC_pool.tile([128, 512], F32, tag="psC")
                                for k in range(nKD):
                                    nc.tensor.matmul(
                                        ps,
                                        out1_tiles[k][:, tm * 128 : (tm + 1) * 128],
                                        w1b[:, k, :],
                                        start=(k == 0),
                                        stop=(k == nKD - 1),
                                    )
                                oc_t = outC_pool.tile([128, 512], F32, tag="oct")
                                nc.vector.tensor_add(oc_t, ps, o2t)
                                nc.scalar.dma_start(
                                    out=outd[
                                        tm * 128 : (tm + 1) * 128,
                                        oc * 512 : (oc + 1) * 512,
                                    ],
                                    in_=oc_t,
                                )

    nc.finalize()
    return nc


def _r22(x):
    """Round fp32 to fp22 (13 explicit mantissa bits) so the PE's float32r
    truncation is exact -> unbiased vs the fp32 reference."""
    u = np.ascontiguousarray(x, dtype=np.float32).view(np.uint32)
    u = (u + np.uint32(0x200)) & np.uint32(0xFFFFFC00)
    return u.view(np.float32)


def make_in_maps(src, tgt, mask, W, b):
    src = np.asarray(src, dtype=np.float32)
    tgt = np.asarray(tgt, dtype=np.float32)
    mask = np.asarray(mask, dtype=np.int32)
    W = np.asarray(W, dtype=np.float32)
    b = np.asarray(b, dtype=np.float32)

    w1t_ = _r22(np.ascontiguousarray(W[:, :D].T))
    w2t_ = _r22(np.ascontiguousarray(W[:, D:].T))
    bias_ = np.ascontiguousarray(b.reshape(1, O))

    in_maps = []
    for bi in range(B):
        in_maps.append(
            {
                "srcT": _r22(np.ascontiguousarray(src[bi].T)),
                "srcN": _r22(src[bi]),
                "tgtT": _r22(np.ascontiguousarray(tgt[bi].T)),
                "maskT": np.ascontiguousarray(
                    mask[:, bi, :].T.astype(np.float32) * np.float32(NEG_BIG)
                ).astype(ml_dtypes.bfloat16),
                "w1t": w1t_,
                "w2t": w2t_,
                "biasd": bias_,
            }
        )
    return in_maps


_PROGRAM = None


def _get_program():
    global _PROGRAM
    if _PROGRAM is None:
        _PROGRAM = build_program()
    return _PROGRAM


def run(inputs, trace=False):
    nc = _get_program()
    in_maps = make_in_maps(**inputs)
    res = run_bass_kernel_spmd(nc, in_maps, list(range(N_CORES)), trace=trace)
    output = np.stack([res.results[i]["outd"] for i in range(B)])
    weight = np.stack([np.ascontiguousarray(res.results[i]["wout"].T) for i in range(B)])
    return (output, weight), res


def kernel(**inputs):
    outs, _ = run(inputs, trace=False)
    return outs


# ---------------------------------------------------------------- benchmarking
def bench(inputs, iters=8, reps=1, k_pipe=16):
    """Steady-state per-dispatch timing with device-resident inputs.

    Compiles the SPMD program once, stages all per-core inputs on the 8
    devices, then times repeated executions (no donation, so buffers stay
    valid). Returns (sorted per-call seconds, outputs_of_last_call).
    """
    import time

    import jax
    from jax.experimental.shard_map import shard_map
    from jax.sharding import Mesh, PartitionSpec

    from concourse import bass2jax as b2j

    b2j.install_neuronx_cc_hook()

    nc = _get_program() if reps == 1 else build_program(reps=reps)
    in_maps = make_in_maps(**inputs)
    n_cores = N_CORES

    in_names, out_names, out_avals, zero_outs = [], [], [], []
    partition_name = nc.partition_id_tensor.name if nc.partition_id_tensor else None
    for alloc in nc.m.functions[0].allocations:
        if not isinstance(alloc, mybir.MemoryLocationSet):
            continue
        name = alloc.memorylocations[0].name
        if alloc.kind == "ExternalInput":
            if name != partition_name:
                in_names.append(name)
        elif alloc.kind == "ExternalOutput":
            out_names.append(name)
            shape = tuple(alloc.tensor_shape)
            dtype = mybir.dt.np(alloc.dtype)
            out_avals.append(jax.core.ShapedArray(shape, dtype))
            zero_outs.append(np.zeros(shape, dtype))
    n_params = len(in_names)
    all_in_names = list(in_names) + list(out_names)
    if partition_name is not None:
        all_in_names.append(partition_name)

    def _body(*args):
        operands = list(args)
        if partition_name is not None:
            operands.append(b2j.partition_id_tensor())
        outs = b2j._bass_exec_p.bind(
            *operands,
            out_avals=tuple(out_avals),
            in_names=tuple(all_in_names),
            out_names=tuple(out_names),
            lowering_input_output_aliases=(),
            sim_require_finite=True,
            sim_require_nnan=True,
            nc=nc,
        )
        return tuple(outs)

    devices = jax.devices()[:n_cores]
    mesh = Mesh(np.asarray(devices), ("core",))
    in_specs = (PartitionSpec("core"),) * (n_params + len(out_names))
    out_specs = (PartitionSpec("core"),) * len(out_names)
    sharded = jax.jit(
        shard_map(_body, mesh=mesh, in_specs=in_specs, out_specs=out_specs,
                  check_rep=False),
        keep_unused=True,
    )
    sharding = jax.sharding.NamedSharding(mesh, PartitionSpec("core"))
    concat_in = [
        jax.device_put(
            np.concatenate([np.asarray(in_maps[c][nm]) for c in range(n_cores)], 0),
            sharding,
        )
        for nm in in_names
    ]
    concat_zeros = [
        jax.device_put(np.zeros((n_cores * z.shape[0], *z.shape[1:]), z.dtype), sharding)
        for z in zero_outs
    ]
    # warmup (compile + first exec)
    outs = sharded(*concat_in, *concat_zeros)
    jax.block_until_ready(outs)
    times = []
    for _ in range(iters):
        t0 = time.perf_counter()
        outs = sharded(*concat_in, *concat_zeros)
        jax.block_until_ready(outs)
        times.append(time.perf_counter() - t0)
    # pipelined: K async dispatches, block once -> slope amortizes RPC RTT
    K_PIPE = k_pipe
    jax.block_until_ready(sharded(*concat_in, *concat_zeros))
    t0 = time.perf_counter()
    last = None
    for _ in range(K_PIPE):
        last = sharded(*concat_in, *concat_zeros)
    jax.block_until_ready(last)
    t_k = time.perf_counter() - t0
    t0 = time.perf_counter()
    jax.block_until_ready(sharded(*concat_in, *concat_zeros))
    t_1 = time.perf_counter() - t0
    pipe_slope = (t_k - t_1) / (K_PIPE - 1)
    results = [
        {nm: np.asarray(outs[i]).reshape(n_cores, *out_avals[i].shape)[c]
         for i, nm in enumerate(out_names)}
        for c in range(n_cores)
    ]
    return (sorted(times), pipe_slope, t_k, t_1), results
